# revision 19
# baseline (speedup 1.0000x reference)
"""Tensor-parallel Llama MHA kernel for 8 TRN2 NeuronCores.

Problem: B=2, S=2048, HIDDEN=2048, 16 heads x head_dim 128, fp32, RoPE + causal.

Sharding: 8 cores = 2 (batch) x 4 (head groups of 4 heads).  Each core computes
q/k/v projections for its 4 heads, flash-style causal attention, and a partial
o_proj (attn_out_heads @ Wo[:, heads].T).  The full output is the sum of the 4
head-group partials per batch element, done on the host after gather.

Device kernel design (per core):
  - Matmul operands in bf16 (full 1 col/cycle PE rate; fp32/fp32r stream at
    half rate), fp32 PSUM accumulation, fp32 output.
  - All inputs are pre-swizzled on the host into the exact SBUF layout
    ([128 partitions, flat free dim]) so every DMA is per-partition
    contiguous (minimal descriptor count, fast HWDGE issue).
  - x.T fully SBUF-resident in bf16; weight half-panels double buffered;
    q-projection weights loaded first so the PE starts early.
  - RoPE: rotate_half via one extra 128-contraction matmul against a constant
    permutation matrix; combined with cos/sin on DVE.  The 1/sqrt(d) score
    scale is folded into the exp activation.
  - Attention (per head, per 512-query chunk): S.T blocks [k=128, q=512],
    software-pipelined one k-block pair ahead of the exp->mask->AV chain so
    the PE never waits for ScalarE; exp on ScalarE (PSUM->SBUF bf16); causal
    masks on diagonal blocks via DVE multiply; O.T [d, q] and column-sums l
    accumulated in PSUM (ones-matrix matmul for l); normalize by 1/l on DVE.
  - o_proj lags one query chunk behind attention so its matmuls never wait
    on the normalization chain.
  - Post pass: TRN2 instructions carry at most one sync wait; excess waits
    are peeled onto same-engine event-semaphore instructions.
"""

import math

import numpy as np

HIDDEN = 2048
NUM_HEADS = 16
HEAD_DIM = 128
BATCH = 2
SEQ = 2048
ROPE_BASE = 10000.0

N_CORES = 8
N_HGROUPS = N_CORES // BATCH          # 4 head-groups
H_LOCAL = NUM_HEADS // N_HGROUPS      # 4 heads per core
D = HEAD_DIM                          # 128
SQ = 512                              # query chunk (free dim of S.T blocks)
KB = 128                              # key block (partition dim of S.T blocks)


def build_bass(seq=SEQ, hid=HIDDEN, h_local=H_LOCAL):
    """Build the single-core Bass program (SPMD: same program on all cores)."""
    import concourse.bass as bass
    import concourse.tile as tile
    from concourse import mybir

    f32 = mybir.dt.float32
    bf16 = mybir.dt.bfloat16
    EXP = mybir.ActivationFunctionType.Exp

    n_qc = seq // SQ                  # query chunks
    n_kc = hid // 128                 # hidden (contraction) chunks
    n_sc = seq // SQ                  # seq chunks of 512
    n_ms = SQ // 128                  # 128-row subchunks in a 512 chunk
    n_on = hid // SQ                  # output col chunks of 512
    m_local = h_local                 # one M-chunk of 128 per head (d=128)
    M = h_local * D                   # projection output width
    isqrt_d = 1.0 / math.sqrt(D)

    nc = bass.Bass(target_bir_lowering=False, trn_type="TRN2")

    # ---- DRAM I/O: host pre-swizzled to [128, flat] layouts, bf16 ----
    xS = nc.dram_tensor("xS", [128, n_kc * seq], bf16, kind="ExternalInput")
    wqS = nc.dram_tensor("wqS", [128, n_kc * M], bf16, kind="ExternalInput")
    wkS = nc.dram_tensor("wkS", [128, n_kc * M], bf16, kind="ExternalInput")
    wvS = nc.dram_tensor("wvS", [128, n_kc * M], bf16, kind="ExternalInput")
    woS = nc.dram_tensor("woS", [128, h_local * hid], bf16, kind="ExternalInput")
    cosT = nc.dram_tensor("cosT", [D, seq], bf16, kind="ExternalInput")
    sinT = nc.dram_tensor("sinT", [D, seq], bf16, kind="ExternalInput")
    rotT = nc.dram_tensor("rotT", [D, D], bf16, kind="ExternalInput")
    maskS = nc.dram_tensor("maskS", [128, (SQ // KB) * SQ], bf16, kind="ExternalInput")
    onesd = nc.dram_tensor("ones", [128, 128], bf16, kind="ExternalInput")
    out = nc.dram_tensor("out", [seq, hid], f32, kind="ExternalOutput")

    with tile.TileContext(nc) as tc:
        with (
            tc.tile_pool(name="persist", bufs=1) as persist,
            tc.tile_pool(name="psum", bufs=1, space="PSUM") as psum,
        ):
            # persistent SBUF tensors
            q_sb = persist.tile([128, h_local, seq], bf16)    # [d, head, seq]
            k_sb = persist.tile([128, h_local, seq], bf16)    # [d, head, seq]
            v_sb = persist.tile([128, seq // 128, M], bf16)   # [s%128, schunk, h*d]
            ones_sb = persist.tile([128, 128], bf16)
            m_sb = persist.tile([128, SQ // KB, SQ], bf16)    # diag masks [kk, j, qq]

            # ================= Phase 1-2: projections + RoPE =================
            with tc.tile_pool(name="proj", bufs=1) as proj:
                n_half = n_kc // 2  # hidden chunks per W half-panel

                def load_w_halves(w_dram):
                    halves = []
                    for half in range(2):
                        w_half = proj.tile(
                            [128, n_half, M], bf16, tag="w_half", bufs=2
                        )
                        nc.sync.dma_start(
                            out=w_half,
                            in_=w_dram[
                                :, half * n_half * M : (half + 1) * n_half * M
                            ].rearrange("p (kc m) -> p kc m", m=M),
                        )
                        halves.append(w_half)
                    return halves

                # q weights first so the PE can start ASAP
                w_q_halves = load_w_halves(wqS)

                # whole x.T resident in bf16: [128, kc, seq]
                x_res = proj.tile([128, n_kc, seq], bf16)
                for kc2 in range(0, n_kc, 2):
                    nc.sync.dma_start(
                        out=x_res[:, kc2 : kc2 + 2, :],
                        in_=xS[:, kc2 * seq : (kc2 + 2) * seq].rearrange(
                            "p (kc s) -> p kc s", s=seq
                        ),
                    )

                cos_sb = proj.tile([128, seq], bf16)
                sin_sb = proj.tile([128, seq], bf16)
                rot_sb = proj.tile([128, 128], bf16)
                nc.sync.dma_start(out=cos_sb, in_=cosT[:])
                nc.sync.dma_start(out=sin_sb, in_=sinT[:])
                nc.sync.dma_start(out=rot_sb, in_=rotT[:])
                nc.sync.dma_start(out=ones_sb, in_=onesd[:])
                nc.sync.dma_start(
                    out=m_sb,
                    in_=maskS[:].rearrange("p (j q) -> p j q", q=SQ),
                )

                for proj_i, (w_dram, dst, is_v) in enumerate(
                    (
                        (wqS, q_sb, False),
                        (wkS, k_sb, False),
                        (wvS, v_sb, True),
                    )
                ):
                    w_halves = w_q_halves if proj_i == 0 else load_w_halves(w_dram)

                    for n in range(n_sc):
                        # one accumulation group per m-chunk (single PSUM
                        # bank each) so evacuation overlaps the next group
                        for m in range(m_local if not is_v else n_ms):
                            ps_m = psum.tile([128, SQ], f32, tag="o", bufs=4)
                            for kc_g in range(n_kc):
                                w_half = w_halves[kc_g // n_half]
                                kc = kc_g % n_half
                                start = kc_g == 0
                                stop = kc_g == n_kc - 1
                                if not is_v:
                                    nc.tensor.matmul(
                                        ps_m,
                                        lhsT=w_half[:, kc, m * D : (m + 1) * D],
                                        rhs=x_res[:, kc_g, n * SQ : (n + 1) * SQ],
                                        start=start,
                                        stop=stop,
                                    )
                                else:
                                    nc.tensor.matmul(
                                        ps_m[:, :M],
                                        lhsT=x_res[
                                            :,
                                            kc_g,
                                            n * SQ + m * 128 : n * SQ
                                            + (m + 1) * 128,
                                        ],
                                        rhs=w_half[:, kc, :],
                                        start=start,
                                        stop=stop,
                                    )
                            if is_v:
                                nc.scalar.copy(
                                    out=v_sb[:, n * n_ms + m, :],
                                    in_=ps_m[:, :M],
                                )
                            else:
                                # RoPE for head m of this seq chunk
                                qraw = proj.tile([128, SQ], bf16, tag="qraw", bufs=3)
                                nc.scalar.copy(out=qraw, in_=ps_m)
                                rh = psum.tile([128, SQ], f32, tag="o", bufs=4)
                                nc.tensor.matmul(
                                    rh,
                                    lhsT=rot_sb,
                                    rhs=qraw,
                                    start=True,
                                    stop=True,
                                )
                                dstv = dst[:, m, n * SQ : (n + 1) * SQ]
                                tmp = proj.tile([128, SQ], bf16, tag="tmp", bufs=3)
                                nc.vector.tensor_mul(
                                    tmp, rh, sin_sb[:, n * SQ : (n + 1) * SQ]
                                )
                                nc.vector.tensor_mul(
                                    dstv,
                                    qraw,
                                    cos_sb[:, n * SQ : (n + 1) * SQ],
                                )
                                nc.vector.tensor_add(dstv, dstv, tmp)

            # ================= Phase 3: attention + o_proj =================
            with tc.tile_pool(name="attn", bufs=1) as attn:
                wo_sb = attn.tile([128, h_local, hid], bf16)  # [d, head, hid]
                nc.sync.dma_start(
                    out=wo_sb,
                    in_=woS[:].rearrange("p (h n) -> p h n", n=hid),
                )

                def o_proj(qc, ot_tiles):
                    # o_proj for query chunk qc: rows qc*512 .. qc*512+511
                    for ms in range(n_ms):
                        for on in range(n_on):
                            oo = psum.tile([128, SQ], f32, tag="o", bufs=4)
                            for h in range(h_local):
                                nc.tensor.matmul(
                                    oo,
                                    lhsT=ot_tiles[h][:, ms * 128 : (ms + 1) * 128],
                                    rhs=wo_sb[:, h, on * SQ : (on + 1) * SQ],
                                    start=(h == 0),
                                    stop=(h == h_local - 1),
                                )
                            out_t = attn.tile([128, SQ], f32, tag="out_t", bufs=4)
                            nc.scalar.copy(out=out_t, in_=oo)
                            nc.sync.dma_start(
                                out=out[
                                    qc * SQ + ms * 128 : qc * SQ + (ms + 1) * 128,
                                    on * SQ : (on + 1) * SQ,
                                ],
                                in_=out_t,
                            )

                pend_ot = {}

                # Software pipeline: the exp for a k-block pair is emitted
                # right after its S matmuls (so ScalarE starts immediately);
                # the mask/AV/l consumption runs one pair behind, so the PE
                # always has the next pair's S matmuls to chew on while the
                # exp completes.
                def consume(st):
                    (kb0, p_sb, o_ps, l_ps, h, qc, n_kb) = st
                    j0 = kb0 - (SQ // KB) * qc
                    if j0 >= 0:
                        # diagonal pair: mask both halves in one DVE multiply
                        # (mask tiles j0, j0+1 are contiguous in m_sb)
                        nc.vector.tensor_mul(
                            p_sb,
                            p_sb,
                            m_sb[:, j0 : j0 + 2, :].rearrange("p j q -> p (j q)"),
                        )
                    for p in range(2):
                        kb = kb0 + p
                        nc.tensor.matmul(
                            o_ps,
                            lhsT=v_sb[:, kb, h * D : (h + 1) * D],
                            rhs=p_sb[:, p * 512 : (p + 1) * 512],
                            start=(kb == 0),
                            stop=(kb == n_kb - 1),
                            skip_group_check=True,
                        )
                        nc.tensor.matmul(
                            l_ps,
                            lhsT=ones_sb,
                            rhs=p_sb[:, p * 512 : (p + 1) * 512],
                            start=(kb == 0),
                            stop=(kb == n_kb - 1),
                            skip_group_check=True,
                        )
                    if kb0 + 2 == n_kb:
                        # last pair of (h, qc): normalize ot = o_ps * (1/l)
                        linv = attn.tile([128, SQ], f32, tag="linv", bufs=2)
                        nc.vector.reciprocal(linv, l_ps)
                        ot = attn.tile([128, SQ], bf16, tag="ot", bufs=8)
                        nc.vector.tensor_mul(ot, o_ps, linv)
                        pend_ot[(qc, h)] = ot

                pend = None
                prev_qc = None
                for qc in range(n_qc):
                    for h in range(h_local):
                        n_kb = (qc + 1) * (SQ // KB)  # causal: blocks 0..n_kb-1
                        o_ps = psum.tile([128, SQ], f32, tag="o", bufs=4)
                        l_ps = psum.tile([128, SQ], f32, tag="o", bufs=4)
                        q_rhs = q_sb[:, h, qc * SQ : (qc + 1) * SQ]
                        for kbp in range(n_kb // 2):
                            kb0 = 2 * kbp
                            s_ps = psum.tile([128, 1024], f32, tag="s", bufs=2)
                            for p in range(2):
                                kb = kb0 + p
                                nc.tensor.matmul(
                                    s_ps[:, p * 512 : (p + 1) * 512],
                                    lhsT=k_sb[:, h, kb * KB : (kb + 1) * KB],
                                    rhs=q_rhs,
                                    start=True,
                                    stop=True,
                                )
                            p_sb = attn.tile([128, 1024], bf16, tag="p_sb", bufs=4)
                            nc.scalar.activation(p_sb, s_ps, EXP, scale=isqrt_d)
                            if pend is not None:
                                consume(pend)
                            pend = (kb0, p_sb, o_ps, l_ps, h, qc, n_kb)
                    # flush the pipeline, then emit the previous chunk's
                    # o_proj (its out_t copies run on the otherwise-idle
                    # ScalarE; DVE stays free for the normalization chain)
                    if pend is not None:
                        consume(pend)
                        pend = None
                    if prev_qc is not None:
                        o_proj(
                            prev_qc,
                            [pend_ot[(prev_qc, h)] for h in range(h_local)],
                        )
                    prev_qc = qc
                o_proj(prev_qc, [pend_ot[(prev_qc, h)] for h in range(h_local)])

    # Finalize (assigns semaphore waits), then legalize: TRN2 instructions
    # accept only ONE sync wait each (EventSemaphore: two, InstISA: zero).
    nc.to_json_bytes()
    _legalize_waits(nc, mybir)
    return nc


def _legalize_waits(nc, mybir):
    """TRN2 instructions carry at most ONE sync wait (InstEventSemaphore:
    two; raw InstISA: none).  Peel excess waits onto event-semaphore
    instructions inserted immediately before, on the same engine sequencer
    (program order keeps the semantics)."""
    nfix = 0
    for f in nc.m.functions:
        for blk in f.blocks:
            insts = list(blk.instructions)
            out = []
            changed = False
            for inst in insts:
                si = getattr(inst, "sync_info", None)
                waits = list(si.on_wait) if si is not None and si.on_wait else []
                tname = type(inst).__name__
                limit = 2 if tname == "InstEventSemaphore" else (
                    0 if tname == "InstISA" else 1
                )
                if len(waits) > limit:
                    keep, excess = waits[:limit], waits[limit:]
                    for k in range(0, len(excess), 2):
                        es = mybir.InstEventSemaphore(
                            name=f"I-waitfix-{nfix}", ins=[], outs=[]
                        )
                        nfix += 1
                        es.engine = inst.engine
                        es.sync_info = mybir.SyncInfo(
                            on_wait=list(excess[k : k + 2]), on_update=[]
                        )
                        nc.register_instruction(es)
                        out.append(es)
                    inst.sync_info = mybir.SyncInfo(
                        on_wait=keep, on_update=list(si.on_update or [])
                    )
                    changed = True
                out.append(inst)
            if changed:
                blk.instructions = out
    return nfix


# ---------------------------------------------------------------------------
# Host-side input prep
# ---------------------------------------------------------------------------

def _rope_cache_np(seq, d):
    inv_freq = 1.0 / (ROPE_BASE ** (np.arange(0, d, 2, dtype=np.float32) / d))
    pos = np.arange(seq, dtype=np.float32)
    rot = pos[:, None] * inv_freq[None, :].astype(np.float32)
    theta = np.concatenate([rot, rot], axis=-1)  # [s, d]
    return np.cos(theta).astype(np.float32), np.sin(theta).astype(np.float32)


def _rot_matrix_np(d):
    """lhsT for rotate_half: (rotT.T @ q) == rotate_half(q)."""
    h = d // 2
    RT = np.zeros((d, d), dtype=np.float32)
    RT[np.arange(h) + h, np.arange(h)] = -1.0  # out[i] = -q[i+h], i < h
    RT[np.arange(h), np.arange(h) + h] = 1.0   # out[i] = q[i-h],  i >= h
    return RT


def _mask_tiles_np():
    """mask[j, kk, qq] = 1.0 if KB*j + kk <= qq else 0.0."""
    j = np.arange(SQ // KB)[:, None, None]
    kk = np.arange(KB)[None, :, None]
    qq = np.arange(SQ)[None, None, :]
    return (KB * j + kk <= qq).astype(np.float32)


def _swizzle_kc(a2d):
    """[n_kc*128, F] -> [128, n_kc*F] (partition-contiguous SBUF layout)."""
    n_kc = a2d.shape[0] // 128
    return np.ascontiguousarray(
        a2d.reshape(n_kc, 128, a2d.shape[1]).transpose(1, 0, 2).reshape(128, -1)
    )


def make_in_maps(hidden_states, Wq, Wk, Wv, Wo):
    import ml_dtypes

    bf = ml_dtypes.bfloat16
    cos, sin = _rope_cache_np(SEQ, D)
    cosT = np.ascontiguousarray(cos.T).astype(bf)
    sinT = np.ascontiguousarray(sin.T).astype(bf)
    rotT = _rot_matrix_np(D).astype(bf)
    mask = _mask_tiles_np()  # [4, 128, 512]
    maskS = np.ascontiguousarray(
        mask.transpose(1, 0, 2).reshape(128, -1)
    ).astype(bf)
    ones = np.ones((128, 128), dtype=bf)

    in_maps = []
    for core in range(N_CORES):
        b = core // N_HGROUPS
        g = core % N_HGROUPS
        rs = slice(g * H_LOCAL * D, (g + 1) * H_LOCAL * D)
        in_maps.append(
            {
                "xS": _swizzle_kc(hidden_states[b].T).astype(bf),
                "wqS": _swizzle_kc(Wq[rs, :].T).astype(bf),
                "wkS": _swizzle_kc(Wk[rs, :].T).astype(bf),
                "wvS": _swizzle_kc(Wv[rs, :].T).astype(bf),
                "woS": _swizzle_kc(Wo[:, rs].T).astype(bf),
                "cosT": cosT,
                "sinT": sinT,
                "rotT": rotT,
                "maskS": maskS,
                "ones": ones,
            }
        )
    return in_maps


def combine_outputs(results):
    """results: list of 8 dicts with 'out' [SEQ, HIDDEN] -> [BATCH, SEQ, HIDDEN]."""
    out = np.zeros((BATCH, SEQ, HIDDEN), dtype=np.float32)
    for core, r in enumerate(results):
        b = core // N_HGROUPS
        out[b] += r["out"]
    return out


_CACHE = {}


def run_hw(inputs, trace=False, **kw):
    """Run on 8 NeuronCores; returns (output, BassKernelResults)."""
    from concourse.bass_utils import run_bass_kernel_spmd

    if "nc" not in _CACHE:
        _CACHE["nc"] = build_bass()
    nc = _CACHE["nc"]
    in_maps = make_in_maps(
        np.asarray(inputs["hidden_states"], dtype=np.float32),
        np.asarray(inputs["Wq"], dtype=np.float32),
        np.asarray(inputs["Wk"], dtype=np.float32),
        np.asarray(inputs["Wv"], dtype=np.float32),
        np.asarray(inputs["Wo"], dtype=np.float32),
    )
    res = run_bass_kernel_spmd(
        nc, in_maps, core_ids=list(range(N_CORES)), trace=trace, **kw
    )
    return combine_outputs(res.results), res


def kernel(hidden_states, Wq, Wk, Wv, Wo):
    out, _ = run_hw(
        {
            "hidden_states": hidden_states,
            "Wq": Wq,
            "Wk": Wk,
            "Wv": Wv,
            "Wo": Wo,
        }
    )
    return out


# revision 20
# speedup vs baseline: 1.1106x; 1.1106x over previous
"""Tensor-parallel Llama MHA kernel for 8 TRN2 NeuronCores.

Problem: B=2, S=2048, HIDDEN=2048, 16 heads x head_dim 128, fp32, RoPE + causal.

Sharding: 8 cores = 2 (batch) x 4 (head groups of 4 heads).  Each core computes
q/k/v projections for its 4 heads, flash-style causal attention, and a partial
o_proj (attn_out_heads @ Wo[:, heads].T).  The full output is the sum of the 4
head-group partials per batch element, done on the host after gather.

Device kernel design (per core):
  - Matmul operands in bf16 (full 1 col/cycle PE rate; fp32/fp32r stream at
    half rate), fp32 PSUM accumulation, fp32 output.
  - All inputs are pre-swizzled on the host into the exact SBUF layout
    ([128 partitions, flat free dim]) so every DMA is per-partition
    contiguous (minimal descriptor count, fast HWDGE issue).
  - x.T fully SBUF-resident in bf16; weight half-panels double buffered;
    q-projection weights loaded first so the PE starts early.
  - RoPE: rotate_half via one extra 128-contraction matmul against a constant
    permutation matrix; combined with cos/sin on DVE.  The 1/sqrt(d) score
    scale is folded into the exp activation.
  - Attention (per head, per 512-query chunk): S.T blocks [k=128, q=512],
    software-pipelined one k-block pair ahead of the exp->mask->AV chain so
    the PE never waits for ScalarE; exp on ScalarE (PSUM->SBUF bf16); causal
    masks on diagonal blocks via DVE multiply; O.T [d, q] and column-sums l
    accumulated in PSUM (ones-matrix matmul for l); normalize by 1/l on DVE.
  - o_proj lags one query chunk behind attention so its matmuls never wait
    on the normalization chain.
  - Post pass: TRN2 instructions carry at most one sync wait; excess waits
    are peeled onto same-engine event-semaphore instructions.
"""

import math

import numpy as np

HIDDEN = 2048
NUM_HEADS = 16
HEAD_DIM = 128
BATCH = 2
SEQ = 2048
ROPE_BASE = 10000.0

N_CORES = 8
N_HGROUPS = N_CORES // BATCH          # 4 head-groups
H_LOCAL = NUM_HEADS // N_HGROUPS      # 4 heads per core
D = HEAD_DIM                          # 128
SQ = 512                              # query chunk (free dim of S.T blocks)
KB = 128                              # key block (partition dim of S.T blocks)


def build_bass(seq=SEQ, hid=HIDDEN, h_local=H_LOCAL):
    """Build the single-core Bass program (SPMD: same program on all cores)."""
    import concourse.bass as bass
    import concourse.tile as tile
    from concourse import mybir

    f32 = mybir.dt.float32
    bf16 = mybir.dt.bfloat16
    EXP = mybir.ActivationFunctionType.Exp

    n_qc = seq // SQ                  # query chunks
    n_kc = hid // 128                 # hidden (contraction) chunks
    n_sc = seq // SQ                  # seq chunks of 512
    n_ms = SQ // 128                  # 128-row subchunks in a 512 chunk
    n_on = hid // SQ                  # output col chunks of 512
    m_local = h_local                 # one M-chunk of 128 per head (d=128)
    M = h_local * D                   # projection output width
    isqrt_d = 1.0 / math.sqrt(D)

    nc = bass.Bass(target_bir_lowering=False, trn_type="TRN2")

    # ---- DRAM I/O: host pre-swizzled to [128, flat] layouts, bf16 ----
    xS = nc.dram_tensor("xS", [128, n_kc * seq], bf16, kind="ExternalInput")
    wqS = nc.dram_tensor("wqS", [128, n_kc * M], bf16, kind="ExternalInput")
    wkS = nc.dram_tensor("wkS", [128, n_kc * M], bf16, kind="ExternalInput")
    wvS = nc.dram_tensor("wvS", [128, n_kc * M], bf16, kind="ExternalInput")
    woS = nc.dram_tensor("woS", [128, h_local * hid], bf16, kind="ExternalInput")
    cosT = nc.dram_tensor("cosT", [D, seq], bf16, kind="ExternalInput")
    sinT = nc.dram_tensor("sinT", [D, seq], bf16, kind="ExternalInput")
    rotT = nc.dram_tensor("rotT", [D, D], bf16, kind="ExternalInput")
    maskS = nc.dram_tensor("maskS", [128, (SQ // KB) * SQ], bf16, kind="ExternalInput")
    onesd = nc.dram_tensor("ones", [128, 128], bf16, kind="ExternalInput")
    out = nc.dram_tensor("out", [seq, hid], f32, kind="ExternalOutput")

    with tile.TileContext(nc) as tc:
        with (
            tc.tile_pool(name="persist", bufs=1) as persist,
            tc.tile_pool(name="psum", bufs=1, space="PSUM") as psum,
        ):
            # persistent SBUF tensors
            q_sb = persist.tile([128, h_local, seq], bf16)    # [d, head, seq]
            k_sb = persist.tile([128, h_local, seq], bf16)    # [d, head, seq]
            v_sb = persist.tile([128, seq // 128, M], bf16)   # [s%128, schunk, h*d]
            ones_sb = persist.tile([128, 128], bf16)
            m_sb = persist.tile([128, SQ // KB, SQ], bf16)    # diag masks [kk, j, qq]

            # ================= Phase 1-2: projections + RoPE =================
            with tc.tile_pool(name="proj", bufs=1) as proj:
                n_half = n_kc // 2  # hidden chunks per W half-panel

                def load_w_halves(w_dram):
                    halves = []
                    for half in range(2):
                        w_half = proj.tile(
                            [128, n_half, M], bf16, tag="w_half", bufs=2
                        )
                        nc.sync.dma_start(
                            out=w_half,
                            in_=w_dram[
                                :, half * n_half * M : (half + 1) * n_half * M
                            ].rearrange("p (kc m) -> p kc m", m=M),
                        )
                        halves.append(w_half)
                    return halves

                # q weights first so the PE can start ASAP
                w_q_halves = load_w_halves(wqS)

                # whole x.T resident in bf16: [128, kc, seq]
                x_res = proj.tile([128, n_kc, seq], bf16)
                for kc2 in range(0, n_kc, 2):
                    nc.sync.dma_start(
                        out=x_res[:, kc2 : kc2 + 2, :],
                        in_=xS[:, kc2 * seq : (kc2 + 2) * seq].rearrange(
                            "p (kc s) -> p kc s", s=seq
                        ),
                    )

                cos_sb = proj.tile([128, seq], bf16)
                sin_sb = proj.tile([128, seq], bf16)
                rot_sb = proj.tile([128, 128], bf16)
                nc.sync.dma_start(out=cos_sb, in_=cosT[:])
                nc.sync.dma_start(out=sin_sb, in_=sinT[:])
                nc.sync.dma_start(out=rot_sb, in_=rotT[:])
                nc.sync.dma_start(out=ones_sb, in_=onesd[:])
                nc.sync.dma_start(
                    out=m_sb,
                    in_=maskS[:].rearrange("p (j q) -> p j q", q=SQ),
                )

                for proj_i, (w_dram, dst, is_v) in enumerate(
                    (
                        (wqS, q_sb, False),
                        (wkS, k_sb, False),
                        (wvS, v_sb, True),
                    )
                ):
                    w_halves = w_q_halves if proj_i == 0 else load_w_halves(w_dram)

                    for n in range(n_sc):
                        # PSUM accumulators for this seq chunk, bank-aligned
                        # (one accumulation group per 2KB PSUM bank)
                        n_acc = ((n_ms if is_v else m_local) * 512) // 1024
                        ps = []
                        for t in range(n_acc):
                            ps_t = psum.tile([128, 1024], f32, tag="s", bufs=2)
                            ps.append(ps_t)

                        def acc_slice(i, width):
                            return ps[(i * 512) // 1024][
                                :, (i * 512) % 1024 : (i * 512) % 1024 + width
                            ]

                        for half in range(2):
                            w_half = w_halves[half]
                            for kc in range(n_half):
                                kc_g = half * n_half + kc
                                x_t = x_res[:, kc_g, n * SQ : (n + 1) * SQ]
                                start = kc_g == 0
                                stop = kc_g == n_kc - 1
                                if not is_v:
                                    for m in range(m_local):
                                        nc.tensor.matmul(
                                            acc_slice(m, SQ),
                                            lhsT=w_half[:, kc, m * D : (m + 1) * D],
                                            rhs=x_t,
                                            start=start,
                                            stop=stop,
                                        )
                                else:
                                    for sub in range(n_ms):
                                        nc.tensor.matmul(
                                            acc_slice(sub, M),
                                            lhsT=x_res[
                                                :,
                                                kc_g,
                                                n * SQ + sub * 128 : n * SQ
                                                + (sub + 1) * 128,
                                            ],
                                            rhs=w_half[:, kc, :],
                                            start=start,
                                            stop=stop,
                                        )
                        if is_v:
                            # split evacuation across ScalarE and VectorE so
                            # the PSUM slots free in half the time
                            for sub in range(n_ms):
                                if sub % 2 == 0:
                                    nc.scalar.copy(
                                        out=v_sb[:, n * n_ms + sub, :],
                                        in_=acc_slice(sub, M),
                                    )
                                else:
                                    nc.vector.tensor_copy(
                                        v_sb[:, n * n_ms + sub, :],
                                        acc_slice(sub, M),
                                    )
                        else:
                            # RoPE for the heads of this seq chunk
                            for t in range(n_acc):
                                qraw = proj.tile(
                                    [128, 1024], bf16, tag="qraw", bufs=2
                                )
                                if t % 2 == 0:
                                    nc.scalar.copy(out=qraw, in_=ps[t])
                                else:
                                    nc.vector.tensor_copy(qraw, ps[t])
                                for p in range(2):
                                    m = 2 * t + p
                                    rh = psum.tile([128, 512], f32, tag="o", bufs=4)
                                    nc.tensor.matmul(
                                        rh,
                                        lhsT=rot_sb,
                                        rhs=qraw[:, p * 512 : (p + 1) * 512],
                                        start=True,
                                        stop=True,
                                    )
                                    dstv = dst[:, m, n * SQ : (n + 1) * SQ]
                                    tmp = proj.tile(
                                        [128, 512], bf16, tag="tmp", bufs=3
                                    )
                                    nc.vector.tensor_mul(
                                        tmp, rh, sin_sb[:, n * SQ : (n + 1) * SQ]
                                    )
                                    nc.vector.tensor_mul(
                                        dstv,
                                        qraw[:, p * 512 : (p + 1) * 512],
                                        cos_sb[:, n * SQ : (n + 1) * SQ],
                                    )
                                    nc.vector.tensor_add(dstv, dstv, tmp)

            # ================= Phase 3: attention + o_proj =================
            with tc.tile_pool(name="attn", bufs=1) as attn:
                wo_sb = attn.tile([128, h_local, hid], bf16)  # [d, head, hid]
                nc.sync.dma_start(
                    out=wo_sb,
                    in_=woS[:].rearrange("p (h n) -> p h n", n=hid),
                )

                def o_proj(qc, ot_tiles):
                    # o_proj for query chunk qc: rows qc*512 .. qc*512+511
                    for ms in range(n_ms):
                        for on in range(n_on):
                            oo = psum.tile([128, SQ], f32, tag="o", bufs=4)
                            for h in range(h_local):
                                nc.tensor.matmul(
                                    oo,
                                    lhsT=ot_tiles[h][:, ms * 128 : (ms + 1) * 128],
                                    rhs=wo_sb[:, h, on * SQ : (on + 1) * SQ],
                                    start=(h == 0),
                                    stop=(h == h_local - 1),
                                )
                            out_t = attn.tile([128, SQ], f32, tag="out_t", bufs=4)
                            nc.vector.tensor_copy(out_t, oo)
                            nc.sync.dma_start(
                                out=out[
                                    qc * SQ + ms * 128 : qc * SQ + (ms + 1) * 128,
                                    on * SQ : (on + 1) * SQ,
                                ],
                                in_=out_t,
                            )

                pend_ot = {}

                # Software pipeline: the exp for a k-block pair is emitted
                # right after its S matmuls (so ScalarE starts immediately);
                # the mask/AV/l consumption runs one pair behind, so the PE
                # always has the next pair's S matmuls to chew on while the
                # exp completes.
                def consume(st):
                    (kb0, p_sb, o_ps, l_ps, h, qc, n_kb) = st
                    j0 = kb0 - (SQ // KB) * qc
                    if j0 >= 0:
                        # diagonal pair: mask both halves in one DVE multiply
                        # (mask tiles j0, j0+1 are contiguous in m_sb)
                        nc.vector.tensor_mul(
                            p_sb,
                            p_sb,
                            m_sb[:, j0 : j0 + 2, :].rearrange("p j q -> p (j q)"),
                        )
                    for p in range(2):
                        kb = kb0 + p
                        nc.tensor.matmul(
                            o_ps,
                            lhsT=v_sb[:, kb, h * D : (h + 1) * D],
                            rhs=p_sb[:, p * 512 : (p + 1) * 512],
                            start=(kb == 0),
                            stop=(kb == n_kb - 1),
                            skip_group_check=True,
                        )
                        nc.tensor.matmul(
                            l_ps,
                            lhsT=ones_sb,
                            rhs=p_sb[:, p * 512 : (p + 1) * 512],
                            start=(kb == 0),
                            stop=(kb == n_kb - 1),
                            skip_group_check=True,
                        )
                    if kb0 + 2 == n_kb:
                        # last pair of (h, qc): normalize ot = o_ps * (1/l)
                        linv = attn.tile([128, SQ], f32, tag="linv", bufs=2)
                        nc.vector.reciprocal(linv, l_ps)
                        ot = attn.tile([128, SQ], bf16, tag="ot", bufs=8)
                        nc.vector.tensor_mul(ot, o_ps, linv)
                        pend_ot[(qc, h)] = ot

                pend = None
                prev_qc = None
                for qc in range(n_qc):
                    for h in range(h_local):
                        n_kb = (qc + 1) * (SQ // KB)  # causal: blocks 0..n_kb-1
                        o_ps = psum.tile([128, SQ], f32, tag="o", bufs=4)
                        l_ps = psum.tile([128, SQ], f32, tag="o", bufs=4)
                        q_rhs = q_sb[:, h, qc * SQ : (qc + 1) * SQ]
                        for kbp in range(n_kb // 2):
                            kb0 = 2 * kbp
                            s_ps = psum.tile([128, 1024], f32, tag="s", bufs=2)
                            for p in range(2):
                                kb = kb0 + p
                                nc.tensor.matmul(
                                    s_ps[:, p * 512 : (p + 1) * 512],
                                    lhsT=k_sb[:, h, kb * KB : (kb + 1) * KB],
                                    rhs=q_rhs,
                                    start=True,
                                    stop=True,
                                )
                            p_sb = attn.tile([128, 1024], bf16, tag="p_sb", bufs=4)
                            nc.scalar.activation(p_sb, s_ps, EXP, scale=isqrt_d)
                            if pend is not None:
                                consume(pend)
                            pend = (kb0, p_sb, o_ps, l_ps, h, qc, n_kb)
                    # flush the pipeline, then emit the previous chunk's
                    # o_proj (its out_t copies run on the otherwise-idle
                    # ScalarE; DVE stays free for the normalization chain)
                    if pend is not None:
                        consume(pend)
                        pend = None
                    if prev_qc is not None:
                        o_proj(
                            prev_qc,
                            [pend_ot[(prev_qc, h)] for h in range(h_local)],
                        )
                    prev_qc = qc
                o_proj(prev_qc, [pend_ot[(prev_qc, h)] for h in range(h_local)])

    # Finalize (assigns semaphore waits), then legalize: TRN2 instructions
    # accept only ONE sync wait each (EventSemaphore: two, InstISA: zero).
    nc.to_json_bytes()
    _legalize_waits(nc, mybir)
    return nc


def _legalize_waits(nc, mybir):
    """TRN2 instructions carry at most ONE sync wait (InstEventSemaphore:
    two; raw InstISA: none).  Peel excess waits onto event-semaphore
    instructions inserted immediately before, on the same engine sequencer
    (program order keeps the semantics)."""
    nfix = 0
    for f in nc.m.functions:
        for blk in f.blocks:
            insts = list(blk.instructions)
            out = []
            changed = False
            for inst in insts:
                si = getattr(inst, "sync_info", None)
                waits = list(si.on_wait) if si is not None and si.on_wait else []
                tname = type(inst).__name__
                limit = 2 if tname == "InstEventSemaphore" else (
                    0 if tname == "InstISA" else 1
                )
                if len(waits) > limit:
                    keep, excess = waits[:limit], waits[limit:]
                    for k in range(0, len(excess), 2):
                        es = mybir.InstEventSemaphore(
                            name=f"I-waitfix-{nfix}", ins=[], outs=[]
                        )
                        nfix += 1
                        es.engine = inst.engine
                        es.sync_info = mybir.SyncInfo(
                            on_wait=list(excess[k : k + 2]), on_update=[]
                        )
                        nc.register_instruction(es)
                        out.append(es)
                    inst.sync_info = mybir.SyncInfo(
                        on_wait=keep, on_update=list(si.on_update or [])
                    )
                    changed = True
                out.append(inst)
            if changed:
                blk.instructions = out
    return nfix


# ---------------------------------------------------------------------------
# Host-side input prep
# ---------------------------------------------------------------------------

def _rope_cache_np(seq, d):
    inv_freq = 1.0 / (ROPE_BASE ** (np.arange(0, d, 2, dtype=np.float32) / d))
    pos = np.arange(seq, dtype=np.float32)
    rot = pos[:, None] * inv_freq[None, :].astype(np.float32)
    theta = np.concatenate([rot, rot], axis=-1)  # [s, d]
    return np.cos(theta).astype(np.float32), np.sin(theta).astype(np.float32)


def _rot_matrix_np(d):
    """lhsT for rotate_half: (rotT.T @ q) == rotate_half(q)."""
    h = d // 2
    RT = np.zeros((d, d), dtype=np.float32)
    RT[np.arange(h) + h, np.arange(h)] = -1.0  # out[i] = -q[i+h], i < h
    RT[np.arange(h), np.arange(h) + h] = 1.0   # out[i] = q[i-h],  i >= h
    return RT


def _mask_tiles_np():
    """mask[j, kk, qq] = 1.0 if KB*j + kk <= qq else 0.0."""
    j = np.arange(SQ // KB)[:, None, None]
    kk = np.arange(KB)[None, :, None]
    qq = np.arange(SQ)[None, None, :]
    return (KB * j + kk <= qq).astype(np.float32)


def _swizzle_kc(a2d):
    """[n_kc*128, F] -> [128, n_kc*F] (partition-contiguous SBUF layout)."""
    n_kc = a2d.shape[0] // 128
    return np.ascontiguousarray(
        a2d.reshape(n_kc, 128, a2d.shape[1]).transpose(1, 0, 2).reshape(128, -1)
    )


def make_in_maps(hidden_states, Wq, Wk, Wv, Wo):
    import ml_dtypes

    bf = ml_dtypes.bfloat16
    cos, sin = _rope_cache_np(SEQ, D)
    cosT = np.ascontiguousarray(cos.T).astype(bf)
    sinT = np.ascontiguousarray(sin.T).astype(bf)
    rotT = _rot_matrix_np(D).astype(bf)
    mask = _mask_tiles_np()  # [4, 128, 512]
    maskS = np.ascontiguousarray(
        mask.transpose(1, 0, 2).reshape(128, -1)
    ).astype(bf)
    ones = np.ones((128, 128), dtype=bf)

    in_maps = []
    for core in range(N_CORES):
        b = core // N_HGROUPS
        g = core % N_HGROUPS
        rs = slice(g * H_LOCAL * D, (g + 1) * H_LOCAL * D)
        in_maps.append(
            {
                "xS": _swizzle_kc(hidden_states[b].T).astype(bf),
                "wqS": _swizzle_kc(Wq[rs, :].T).astype(bf),
                "wkS": _swizzle_kc(Wk[rs, :].T).astype(bf),
                "wvS": _swizzle_kc(Wv[rs, :].T).astype(bf),
                "woS": _swizzle_kc(Wo[:, rs].T).astype(bf),
                "cosT": cosT,
                "sinT": sinT,
                "rotT": rotT,
                "maskS": maskS,
                "ones": ones,
            }
        )
    return in_maps


def combine_outputs(results):
    """results: list of 8 dicts with 'out' [SEQ, HIDDEN] -> [BATCH, SEQ, HIDDEN]."""
    out = np.zeros((BATCH, SEQ, HIDDEN), dtype=np.float32)
    for core, r in enumerate(results):
        b = core // N_HGROUPS
        out[b] += r["out"]
    return out


_CACHE = {}


def run_hw(inputs, trace=False, **kw):
    """Run on 8 NeuronCores; returns (output, BassKernelResults)."""
    from concourse.bass_utils import run_bass_kernel_spmd

    if "nc" not in _CACHE:
        _CACHE["nc"] = build_bass()
    nc = _CACHE["nc"]
    in_maps = make_in_maps(
        np.asarray(inputs["hidden_states"], dtype=np.float32),
        np.asarray(inputs["Wq"], dtype=np.float32),
        np.asarray(inputs["Wk"], dtype=np.float32),
        np.asarray(inputs["Wv"], dtype=np.float32),
        np.asarray(inputs["Wo"], dtype=np.float32),
    )
    res = run_bass_kernel_spmd(
        nc, in_maps, core_ids=list(range(N_CORES)), trace=trace, **kw
    )
    return combine_outputs(res.results), res


def kernel(hidden_states, Wq, Wk, Wv, Wo):
    out, _ = run_hw(
        {
            "hidden_states": hidden_states,
            "Wq": Wq,
            "Wk": Wk,
            "Wv": Wv,
            "Wo": Wo,
        }
    )
    return out
